# revision 9
# baseline (speedup 1.0000x reference)
"""Trainium2 Bass kernel v2: multi-head self-attention with RoPE (B=2,
S=2048, d_model=1024, 16 heads, causal) distributed over 8 NeuronCores.

Sharding: core c handles batch b = c//4 and head-group c%4 (4 heads as two
pairs). Each core computes its partial output projection over its 256
v-channels; the host sums the 4 partials per batch element.

v2 vs baseline:
- bf16 matmul operands (fp32 psum accumulate)
- merged even/odd head-dim layout: score matmuls contract 64-deep in one
  pass (half the passes of the e/o-split baseline)
- attention output accumulated as [q, v] via per-head N=65 matmuls with an
  augmented ones-column carrying the softmax denominator; normalization is
  a per-partition broadcast multiply; a PE transpose restores [v, q] for
  the output projection
- heads processed in pairs so PSUM fits: scores double-buffered (2x2
  banks), av/transpose tiles 4x1 bank
"""

from contextlib import ExitStack

import numpy as np

import concourse.bass as bass
import concourse.bacc as bacc
import concourse.tile as tile
import concourse.mybir as mybir

F32 = mybir.dt.float32
BF16 = mybir.dt.bfloat16
EXP = mybir.ActivationFunctionType.Exp

D = 1024          # d_model
HPC = 4           # heads per core
DK = 64
DL = HPC * DK     # 256 local v-channels
QB = 512          # q block
KB = 128          # k tile
NDC = D // 128    # 8 contraction chunks
NP = 2            # head pairs per core


def build_nc(S=2048, mm_dtype="bf16", repeat=1):
    assert mm_dtype == "bf16"
    nq = S // QB
    nk = S // KB
    nc = bacc.Bacc("TRN2", target_bir_lowering=False, debug=False,
                   enable_asserts=True)
    xT = nc.dram_tensor("xT", [128, NDC * S], BF16, kind="ExternalInput").ap()
    wqeT = nc.dram_tensor("wqeT", [128, NDC * 128], BF16, kind="ExternalInput").ap()
    wqoT = nc.dram_tensor("wqoT", [128, NDC * 128], BF16, kind="ExternalInput").ap()
    wkeT = nc.dram_tensor("wkeT", [128, NDC * 128], BF16, kind="ExternalInput").ap()
    wkoT = nc.dram_tensor("wkoT", [128, NDC * 128], BF16, kind="ExternalInput").ap()
    wvT = nc.dram_tensor("wvT", [128, NDC * DL], BF16, kind="ExternalInput").ap()
    woT = nc.dram_tensor("woT", [128, (DL // 128) * D], BF16,
                         kind="ExternalInput").ap()
    cosb = nc.dram_tensor("cosb", [128, S], BF16, kind="ExternalInput").ap()
    sinb = nc.dram_tensor("sinb", [128, S], BF16, kind="ExternalInput").ap()
    trim = nc.dram_tensor("trim", [KB, KB], BF16, kind="ExternalInput").ap()
    idnt = nc.dram_tensor("idnt", [128, 128], BF16, kind="ExternalInput").ap()
    out = nc.dram_tensor("out", [S, D], BF16, kind="ExternalOutput").ap()

    with tile.TileContext(nc) as tc, ExitStack() as ctx, \
            nc.allow_low_precision(reason="bf16 matmul operands, fp32 psum"):
        const = ctx.enter_context(tc.tile_pool(name="const", bufs=1))
        xtp = ctx.enter_context(tc.tile_pool(name="xtp", bufs=2))
        qkp = ctx.enter_context(tc.tile_pool(name="qkp", bufs=2))
        vap = ctx.enter_context(tc.tile_pool(name="vap", bufs=2))
        qp = ctx.enter_context(tc.tile_pool(name="qp", bufs=2))
        rtp = ctx.enter_context(tc.tile_pool(name="rtp", bufs=6))
        ptp = ctx.enter_context(tc.tile_pool(name="ptp", bufs=8))
        smp = ctx.enter_context(tc.tile_pool(name="smp", bufs=4))
        otp = ctx.enter_context(tc.tile_pool(name="otp", bufs=4))
        osp = ctx.enter_context(tc.tile_pool(name="osp", bufs=3))
        # PSUM: "sc" 2-bank tiles (scores / proj / outproj) x2 bufs = 4
        # banks; "av" 1-bank tiles (av halves + transposes) x4 bufs.
        pp = ctx.enter_context(tc.tile_pool(name="pp", bufs=1, space="PSUM"))

        # ---- constants / weights to SBUF ----
        w_sb = {}

        def load_w(name, ap_, ncol):
            t = const.tile([128, NDC, ncol], BF16, name=f"sb_{name}",
                           tag=f"sb_{name}")
            nc.sync.dma_start(out=t,
                              in_=ap_.rearrange("p (c m) -> p c m", c=NDC))
            w_sb[name] = t

        # startup-critical first: Q weights, then x(0) is emitted by the
        # caller loop right after; remaining weights follow.
        for name, ap_ in (("wqe", wqeT), ("wqo", wqoT)):
            load_w(name, ap_, 128)

        def load_rest_of_consts():
            for name, ap_ in (("wke", wkeT), ("wko", wkoT)):
                load_w(name, ap_, 128)
            cs_sb = const.tile([128, S], BF16, name="sb_cos", tag="sb_cos")
            sn_sb = const.tile([128, S], BF16, name="sb_sin", tag="sb_sin")
            # split per q block so rope(0) waits only on the first slice
            for st in range(S // QB):
                ssl = slice(st * QB, (st + 1) * QB)
                nc.sync.dma_start(out=cs_sb[:, ssl], in_=cosb[:, ssl])
                nc.sync.dma_start(out=sn_sb[:, ssl], in_=sinb[:, ssl])
            load_w("wv", wvT, DL)
            tri_sb = const.tile([KB, KB], BF16, name="sb_tri", tag="sb_tri")
            nc.sync.dma_start(out=tri_sb, in_=trim)
            id_sb = const.tile([128, 128], BF16, name="sb_id", tag="sb_id")
            nc.sync.dma_start(out=id_sb, in_=idnt)
            wo_sb = const.tile([128, DL // 128, D], BF16, name="sb_wo",
                               tag="sb_wo")
            nc.sync.dma_start(out=wo_sb,
                              in_=woT.rearrange("p (c m) -> p c m",
                                                c=DL // 128))
            return wo_sb, cs_sb, sn_sb, tri_sb, id_sb

        def load_x(st):
            ssl = slice(st * QB, (st + 1) * QB)
            t = xtp.tile([128, NDC, QB], BF16, name="xt", tag="xt")
            nc.sync.dma_start(
                out=t,
                in_=xT.rearrange("p (c s) -> p c s", c=NDC)[:, :, ssl])
            return t

        g = {}   # late-loaded consts (wo/cs/sn/tri/id) + per-rep km tiles

        def project_block(st, xts, vaug):
            """QKV projection + rope for s-block st. Returns qm pair tiles.
            Uses the dedicated 1-bank "pj" psum tag so projection matmuls can
            fill PE gaps during the (ACT-bound) attention inner loop."""
            ssl = slice(st * QB, (st + 1) * QB)
            cs = g["cs"][:, ssl]
            sn = g["sn"][:, ssl]
            qm = [qp.tile([128, QB], BF16, name=f"qm{p}", tag=f"qm{p}")
                  for p in range(NP)]
            km = g["km"]
            for wen, dst in (
                    (("wqe", "wqo"), qm),
                    (("wke", "wko"), [km[p][:, ssl] for p in range(NP)])):
                we, wo_ = w_sb[wen[0]], w_sb[wen[1]]
                pse = pp.tile([128, QB], F32, name="ps_e", tag="pj", bufs=1)
                for dc in range(NDC):
                    nc.tensor.matmul(pse, we[:, dc, :], xts[:, dc, :],
                                     start=(dc == 0), stop=(dc == NDC - 1))
                t1 = rtp.tile([128, QB], BF16, name="t1", tag="t1")
                t3 = rtp.tile([128, QB], BF16, name="t3", tag="t3")
                nc.vector.tensor_mul(t1, pse, cs)
                nc.vector.tensor_mul(t3, pse, sn)
                pso_ = pp.tile([128, QB], F32, name="ps_o", tag="pj", bufs=1)
                for dc in range(NDC):
                    nc.tensor.matmul(pso_, wo_[:, dc, :], xts[:, dc, :],
                                     start=(dc == 0), stop=(dc == NDC - 1))
                t2 = rtp.tile([128, QB], BF16, name="t2", tag="t2")
                t4 = rtp.tile([128, QB], BF16, name="t4", tag="t4")
                nc.vector.tensor_mul(t2, pso_, sn)
                nc.vector.tensor_mul(t4, pso_, cs)
                te = rtp.tile([128, QB], BF16, name="te", tag="te")
                nc.vector.tensor_sub(te, t1, t2)
                to = rtp.tile([128, QB], BF16, name="to", tag="to")
                nc.vector.tensor_add(to, t3, t4)
                # merge: head h evens -> rows 64h..+32, odds -> 64h+32..+64
                for h in range(HPC):
                    p, dd = h // 2, h % 2
                    nc.vector.tensor_copy(
                        dst[p][64 * dd:64 * dd + 32, :],
                        te[32 * h:32 * h + 32, :])
                    nc.vector.tensor_copy(
                        dst[p][64 * dd + 32:64 * dd + 64, :],
                        to[32 * h:32 * h + 32, :])
            # V for the 4 k-subtiles of this block
            for ss in range(4):
                kt = st * 4 + ss
                psv = pp.tile([128, DL], F32, name="ps_v", tag="pj", bufs=1)
                for dc in range(NDC):
                    nc.tensor.matmul(
                        psv, xts[:, dc, ss * KB:(ss + 1) * KB],
                        w_sb["wv"][:, dc, :],
                        start=(dc == 0), stop=(dc == NDC - 1))
                for p in range(NP):
                    nc.vector.tensor_copy(
                        vaug[p][:, kt, :, 0:64],
                        psv[:, 128 * p:128 * p + 128].rearrange(
                            "k (h d) -> k h d", h=2))
            return qm

        def attend_pair(qj, p, qm, vaug):
            """Attention for head pair p of q-block qj. Returns tp psum tile
            [128 v, 4 tq, 128 q]: transposed normalized attention output."""
            nki = 4 * qj + 4
            avh = [pp.tile([128, 2, 2, 65], F32, name=f"ps_av{half}",
                           tag="av", bufs=3)
                   for half in range(2)]
            tp = pp.tile([128, 4, 128], BF16, name="ps_tp", tag="av", bufs=3)
            # ot: normalized [q, (t, h, d)] bf16, one slot per qtile
            ot = otp.tile([128, 4, 2, 64], BF16, name="ot", tag="ot")

            def finish_half(half):
                # normalize: ot = av[...,0:64] * (1/den); den at col 64
                av = avh[half]
                rcp = smp.tile([128, 2, 2, 1], F32, name="rcp", tag="rcp")
                nc.vector.reciprocal(
                    rcp, av[:, :, :, 64:65])
                nc.vector.tensor_mul(
                    ot[:, 2 * half:2 * half + 2, :, :],
                    av[:, :, :, 0:64],
                    rcp.broadcast_to([128, 2, 2, 64]))
                # transpose the two qtiles of this half: [q, 2h x 64] -> [v, q]
                # all 4 transposes share one psum bank: single accumulation
                # group (start zeroes the whole 2KB zero region; disjoint
                # slots then accumulate onto zeros)
                for tq in range(2):
                    t = 2 * half + tq
                    nc.tensor.matmul(
                        tp[:, t, :],
                        ot[:, t, :, :].rearrange("q h d -> q (h d)"),
                        g["id"], is_transpose=True,
                        start=(t == 0), stop=(t == 3))

            for ki in range(nki):
                diag = (ki // 4 == qj)
                off = KB * (ki % 4) if diag else 0
                ksl = slice(ki * KB, (ki + 1) * KB)
                sc = pp.tile([128, 2, QB], F32, name="ps_sc", tag="sc", bufs=2)
                for h in range(2):
                    hp = slice(64 * h, 64 * h + 64)
                    nc.tensor.matmul(sc[:, h, off:], g["km"][p][hp, ksl],
                                     qm[hp, off:], start=True, stop=not diag,
                                     tile_position=(64 * h, 0))
                    if diag:
                        # accumulate -1e9 strict-lower-tri(k,q) causal mask:
                        # out += trimT^T @ I; exp then zeroes masked slots
                        nc.tensor.matmul(sc[:, h, off:off + KB], g["tri"],
                                         g["id"], start=False, stop=True)
                pt = ptp.tile([128, 2, QB], BF16, name="pt", tag="pt")
                nc.scalar.activation(out=pt[:, :, off:], in_=sc[:, :, off:],
                                     func=EXP, scale=0.125)
                t0 = max(0, ki - 4 * qj)
                for t in range(t0, 4):
                    half, tq = t // 2, t % 2
                    for h in range(2):
                        # one accumulation group per avh bank: first write
                        # zeroes the whole bank, last (its diag, h=1) stops
                        nc.tensor.matmul(
                            avh[half][:, tq, h, :],
                            pt[:, h, t * KB:(t + 1) * KB],
                            vaug[p][:, ki, h, :],
                            start=(ki == 0 and tq == 0 and h == 0),
                            stop=(ki == 4 * qj + 2 * half + 1 and tq == 1
                                  and h == 1))
                # halves complete as soon as their diagonal ki is done
                if diag and ki % 4 == 1:
                    finish_half(0)
            finish_half(1)
            return tp

        def out_block(qj, otT, wo_sb):
            """Output projection for q block qj from otT[p] = [128 v, 4 tq,
            128 q] bf16 sbuf tiles."""
            for t in range(4):
                pso = pp.tile([128, 2, QB], F32, name="ps_o", tag="sc", bufs=2)
                ost = osp.tile([128, 2, QB], BF16, name="ost", tag="ost")
                for dt_ in range(2):
                    for p in range(NP):
                        nc.tensor.matmul(
                            pso[:, dt_, :], otT[p][:, t, :],
                            wo_sb[:, p, dt_ * QB:(dt_ + 1) * QB],
                            start=(p == 0), stop=(p == NP - 1))
                    nc.vector.tensor_copy(ost[:, dt_, :], pso[:, dt_, :])
                nc.sync.dma_start(
                    out=out[qj * QB + t * KB: qj * QB + (t + 1) * KB, :],
                    in_=ost.rearrange("p a b -> p (a b)"))

        def attend_and_copy(st, p, qm, vaug):
            tp = attend_pair(st, p, qm, vaug)
            ott = otp.tile([128, 4, 128], BF16, name="otT", tag="otT")
            nc.vector.tensor_copy(ott, tp)
            return ott

        wo_box = []
        for _rep in range(repeat):
            vaug = [vap.tile([128, nk, 2, 65], BF16, name=f"vaug{p}",
                             tag=f"vaug{p}") for p in range(NP)]
            for p in range(NP):
                nc.vector.memset(vaug[p][:, :, :, 64], 1.0)
            xts = load_x(0)
            if _rep == 0:
                wo_sb_, cs_, sn_, tri_, id_ = load_rest_of_consts()
                g.update({"cs": cs_, "sn": sn_, "tri": tri_, "id": id_})
                wo_box.append(wo_sb_)
            wo_sb = wo_box[0]
            g["km"] = [qkp.tile([128, S], BF16, name=f"km{p}", tag=f"km{p}")
                       for p in range(NP)]
            qm = project_block(0, xts, vaug)
            pending_out = None
            for st in range(nq):
                otA = attend_and_copy(st, 0, qm[0], vaug)
                if pending_out is not None:
                    out_block(st - 1, pending_out, wo_sb)
                qm_next = None
                if st + 1 < nq:
                    xts = load_x(st + 1)
                    qm_next = project_block(st + 1, xts, vaug)
                otB = attend_and_copy(st, 1, qm[1], vaug)
                pending_out = [otA, otB]
                qm = qm_next
            out_block(nq - 1, pending_out, wo_sb)

    nc.compile()
    return nc


# ---------------- host-side helpers ----------------

def core_slices(core):
    """Global W-row index arrays for a core's sharded weight layout."""
    hg = core % 4
    heads = [4 * hg + h for h in range(HPC)]
    qe_rows = np.concatenate(
        [64 * g + 2 * np.arange(32) for g in heads])          # [128]
    qo_rows = qe_rows + 1
    v_rows = np.concatenate([64 * g + np.arange(64) for g in heads])  # [256]
    return heads, qe_rows, qo_rows, v_rows


def make_in_map(core, x, W_q, W_k, W_v, W_o, positions, theta, S,
                mm_dtype="bf16"):
    import ml_dtypes
    bf = ml_dtypes.bfloat16
    b = core // 4
    _, qe_rows, qo_rows, v_rows = core_slices(core)
    cT = lambda a: np.ascontiguousarray(a.astype(bf))
    pos = np.asarray(positions).astype(np.float32)
    inv_freq = np.float32(theta) ** (
        -np.arange(0, 32, dtype=np.float32) * np.float32(2.0 / DK))
    ang = pos[None, :] * inv_freq[:, None]          # [32, S]
    cosb = np.tile(np.cos(ang), (4, 1)).astype(np.float32)
    sinb = np.tile(np.sin(ang), (4, 1)).astype(np.float32)
    # mask lhsT: out[k,q] += trim[q,k] via matmul with identity rhs;
    # want -1e9 where k > q  ->  trim[q,k] = -1e9 for k > q (strict upper)
    trim = np.triu(np.full((KB, KB), -1e9, np.float32), 1)

    def pmajor(wt):   # [d, ncol] -> [128, (d//128)*ncol] partition-major
        d, ncol = wt.shape
        return wt.reshape(d // 128, 128, ncol).transpose(1, 0, 2).reshape(
            128, (d // 128) * ncol)

    f32c = lambda a: np.ascontiguousarray(np.asarray(a, dtype=np.float32))
    return {
        "xT": cT(np.asarray(x[b]).T.reshape(NDC, 128, S).transpose(1, 0, 2).reshape(128, NDC * S)),
        "wqeT": cT(pmajor(np.asarray(W_q)[qe_rows].T)),
        "wqoT": cT(pmajor(np.asarray(W_q)[qo_rows].T)),
        "wkeT": cT(pmajor(np.asarray(W_k)[qe_rows].T)),
        "wkoT": cT(pmajor(np.asarray(W_k)[qo_rows].T)),
        "wvT": cT(pmajor(np.asarray(W_v)[v_rows].T)),
        "woT": cT(pmajor(np.asarray(W_o)[:, v_rows].T)),
        "cosb": cT(cosb[:, :S]),
        "sinb": cT(sinb[:, :S]),
        "trim": cT(trim),
        "idnt": cT(np.eye(128, dtype=np.float32)),
    }


# ---------------- public entry point ----------------

S_FULL = 2048
MM_DTYPE = "bf16"
_NC_CACHE = {}


def _get_nc():
    if "nc" not in _NC_CACHE:
        _NC_CACHE["nc"] = build_nc(S=S_FULL, mm_dtype=MM_DTYPE)
    return _NC_CACHE["nc"]


def kernel(x, W_q, W_k, W_v, W_o, token_positions, max_seq_len, theta):
    from concourse import bass_utils

    x = np.asarray(x, dtype=np.float32)
    W_q = np.asarray(W_q, dtype=np.float32)
    W_k = np.asarray(W_k, dtype=np.float32)
    W_v = np.asarray(W_v, dtype=np.float32)
    W_o = np.asarray(W_o, dtype=np.float32)
    positions = np.asarray(token_positions)
    theta_f = float(np.asarray(theta))

    nc = _get_nc()
    in_maps = [
        make_in_map(c, x, W_q, W_k, W_v, W_o, positions, theta_f, S_FULL,
                    mm_dtype=MM_DTYPE)
        for c in range(8)
    ]
    res = bass_utils.run_bass_kernel_spmd(nc, in_maps, core_ids=list(range(8)))
    outs = [np.asarray(res.results[c]["out"], dtype=np.float32)
            for c in range(8)]
    full = np.empty((2, S_FULL, 1024), np.float32)
    for b in range(2):
        full[b] = np.sum([outs[4 * b + i] for i in range(4)], axis=0,
                         dtype=np.float32)
    return full


# revision 10
# speedup vs baseline: 1.0575x; 1.0575x over previous
"""Trainium2 Bass kernel v2: multi-head self-attention with RoPE (B=2,
S=2048, d_model=1024, 16 heads, causal) distributed over 8 NeuronCores.

Sharding: core c handles batch b = c//4 and head-group c%4 (4 heads as two
pairs). Each core computes its partial output projection over its 256
v-channels; the host sums the 4 partials per batch element.

v2 vs baseline:
- bf16 matmul operands (fp32 psum accumulate)
- merged even/odd head-dim layout: score matmuls contract 64-deep in one
  pass (half the passes of the e/o-split baseline)
- attention output accumulated as [q, v] via per-head N=65 matmuls with an
  augmented ones-column carrying the softmax denominator; normalization is
  a per-partition broadcast multiply; a PE transpose restores [v, q] for
  the output projection
- heads processed in pairs so PSUM fits: scores double-buffered (2x2
  banks), av/transpose tiles 4x1 bank
"""

from contextlib import ExitStack

import numpy as np

import concourse.bass as bass
import concourse.bacc as bacc
import concourse.tile as tile
import concourse.mybir as mybir

F32 = mybir.dt.float32
BF16 = mybir.dt.bfloat16
EXP = mybir.ActivationFunctionType.Exp

D = 1024          # d_model
HPC = 4           # heads per core
DK = 64
DL = HPC * DK     # 256 local v-channels
QB = 512          # q block
KB = 128          # k tile
NDC = D // 128    # 8 contraction chunks
NP = 2            # head pairs per core


def build_nc(S=2048, mm_dtype="bf16", repeat=1):
    assert mm_dtype == "bf16"
    nq = S // QB
    nk = S // KB
    nc = bacc.Bacc("TRN2", target_bir_lowering=False, debug=False,
                   enable_asserts=True)
    xT = nc.dram_tensor("xT", [D, S], BF16, kind="ExternalInput").ap()
    wqeT = nc.dram_tensor("wqeT", [128, NDC * 128], BF16, kind="ExternalInput").ap()
    wqoT = nc.dram_tensor("wqoT", [128, NDC * 128], BF16, kind="ExternalInput").ap()
    wkeT = nc.dram_tensor("wkeT", [128, NDC * 128], BF16, kind="ExternalInput").ap()
    wkoT = nc.dram_tensor("wkoT", [128, NDC * 128], BF16, kind="ExternalInput").ap()
    wvT = nc.dram_tensor("wvT", [128, NDC * DL], BF16, kind="ExternalInput").ap()
    woT = nc.dram_tensor("woT", [128, (DL // 128) * D], BF16,
                         kind="ExternalInput").ap()
    cosb = nc.dram_tensor("cosb", [128, S], BF16, kind="ExternalInput").ap()
    sinb = nc.dram_tensor("sinb", [128, S], BF16, kind="ExternalInput").ap()
    trim = nc.dram_tensor("trim", [KB, KB], BF16, kind="ExternalInput").ap()
    idnt = nc.dram_tensor("idnt", [128, 128], BF16, kind="ExternalInput").ap()
    out = nc.dram_tensor("out", [S, D], BF16, kind="ExternalOutput").ap()

    with tile.TileContext(nc) as tc, ExitStack() as ctx, \
            nc.allow_low_precision(reason="bf16 matmul operands, fp32 psum"):
        const = ctx.enter_context(tc.tile_pool(name="const", bufs=1))
        xtp = ctx.enter_context(tc.tile_pool(name="xtp", bufs=2 * NDC))
        qkp = ctx.enter_context(tc.tile_pool(name="qkp", bufs=2))
        vap = ctx.enter_context(tc.tile_pool(name="vap", bufs=2))
        qp = ctx.enter_context(tc.tile_pool(name="qp", bufs=2))
        rtp = ctx.enter_context(tc.tile_pool(name="rtp", bufs=6))
        ptp = ctx.enter_context(tc.tile_pool(name="ptp", bufs=8))
        smp = ctx.enter_context(tc.tile_pool(name="smp", bufs=4))
        otp = ctx.enter_context(tc.tile_pool(name="otp", bufs=4))
        osp = ctx.enter_context(tc.tile_pool(name="osp", bufs=3))
        # PSUM: "sc" 2-bank tiles (scores / proj / outproj) x2 bufs = 4
        # banks; "av" 1-bank tiles (av halves + transposes) x4 bufs.
        pp = ctx.enter_context(tc.tile_pool(name="pp", bufs=1, space="PSUM"))

        # ---- constants / weights to SBUF ----
        w_sb = {}

        def load_w(name, ap_, ncol):
            t = const.tile([128, NDC, ncol], BF16, name=f"sb_{name}",
                           tag=f"sb_{name}")
            nc.sync.dma_start(out=t,
                              in_=ap_.rearrange("p (c m) -> p c m", c=NDC))
            w_sb[name] = t

        # startup-critical first: Q weights, then x(0) is emitted by the
        # caller loop right after; remaining weights follow.
        for name, ap_ in (("wqe", wqeT), ("wqo", wqoT)):
            load_w(name, ap_, 128)

        def load_rest_of_consts():
            for name, ap_ in (("wke", wkeT), ("wko", wkoT)):
                load_w(name, ap_, 128)
            cs_sb = const.tile([128, S], BF16, name="sb_cos", tag="sb_cos")
            sn_sb = const.tile([128, S], BF16, name="sb_sin", tag="sb_sin")
            # split per q block so rope(0) waits only on the first slice
            for st in range(S // QB):
                ssl = slice(st * QB, (st + 1) * QB)
                nc.sync.dma_start(out=cs_sb[:, ssl], in_=cosb[:, ssl])
                nc.sync.dma_start(out=sn_sb[:, ssl], in_=sinb[:, ssl])
            load_w("wv", wvT, DL)
            tri_sb = const.tile([KB, KB], BF16, name="sb_tri", tag="sb_tri")
            nc.sync.dma_start(out=tri_sb, in_=trim)
            id_sb = const.tile([128, 128], BF16, name="sb_id", tag="sb_id")
            nc.sync.dma_start(out=id_sb, in_=idnt)
            wo_sb = const.tile([128, DL // 128, D], BF16, name="sb_wo",
                               tag="sb_wo")
            nc.sync.dma_start(out=wo_sb,
                              in_=woT.rearrange("p (c m) -> p c m",
                                                c=DL // 128))
            return wo_sb, cs_sb, sn_sb, tri_sb, id_sb

        def load_x(st):
            ssl = slice(st * QB, (st + 1) * QB)
            xts = []
            for dc in range(NDC):
                t = xtp.tile([128, QB], BF16, name=f"xt{dc}", tag="xt")
                nc.sync.dma_start(out=t, in_=xT[dc * 128:(dc + 1) * 128, ssl])
                xts.append(t)
            return xts

        g = {}   # late-loaded consts (wo/cs/sn/tri/id) + per-rep km tiles

        def project_block(st, xts, vaug):
            """QKV projection + rope for s-block st. Returns qm pair tiles.
            Uses the dedicated 1-bank "pj" psum tag so projection matmuls can
            fill PE gaps during the (ACT-bound) attention inner loop."""
            ssl = slice(st * QB, (st + 1) * QB)
            cs = g["cs"][:, ssl]
            sn = g["sn"][:, ssl]
            qm = [qp.tile([128, QB], BF16, name=f"qm{p}", tag=f"qm{p}")
                  for p in range(NP)]
            km = g["km"]
            for wen, dst in (
                    (("wqe", "wqo"), qm),
                    (("wke", "wko"), [km[p][:, ssl] for p in range(NP)])):
                we, wo_ = w_sb[wen[0]], w_sb[wen[1]]
                pse = pp.tile([128, QB], F32, name="ps_e", tag="pj", bufs=1)
                for dc in range(NDC):
                    nc.tensor.matmul(pse, we[:, dc, :], xts[dc],
                                     start=(dc == 0), stop=(dc == NDC - 1))
                t1 = rtp.tile([128, QB], BF16, name="t1", tag="t1")
                t3 = rtp.tile([128, QB], BF16, name="t3", tag="t3")
                nc.vector.tensor_mul(t1, pse, cs)
                nc.vector.tensor_mul(t3, pse, sn)
                pso_ = pp.tile([128, QB], F32, name="ps_o", tag="pj", bufs=1)
                for dc in range(NDC):
                    nc.tensor.matmul(pso_, wo_[:, dc, :], xts[dc],
                                     start=(dc == 0), stop=(dc == NDC - 1))
                t2 = rtp.tile([128, QB], BF16, name="t2", tag="t2")
                t4 = rtp.tile([128, QB], BF16, name="t4", tag="t4")
                nc.vector.tensor_mul(t2, pso_, sn)
                nc.vector.tensor_mul(t4, pso_, cs)
                te = rtp.tile([128, QB], BF16, name="te", tag="te")
                nc.vector.tensor_sub(te, t1, t2)
                to = rtp.tile([128, QB], BF16, name="to", tag="to")
                nc.vector.tensor_add(to, t3, t4)
                # merge: head h evens -> rows 64h..+32, odds -> 64h+32..+64
                for h in range(HPC):
                    p, dd = h // 2, h % 2
                    nc.vector.tensor_copy(
                        dst[p][64 * dd:64 * dd + 32, :],
                        te[32 * h:32 * h + 32, :])
                    nc.vector.tensor_copy(
                        dst[p][64 * dd + 32:64 * dd + 64, :],
                        to[32 * h:32 * h + 32, :])
            # V for the 4 k-subtiles of this block
            for ss in range(4):
                kt = st * 4 + ss
                psv = pp.tile([128, DL], F32, name="ps_v", tag="pj", bufs=1)
                for dc in range(NDC):
                    nc.tensor.matmul(
                        psv, xts[dc][:, ss * KB:(ss + 1) * KB],
                        w_sb["wv"][:, dc, :],
                        start=(dc == 0), stop=(dc == NDC - 1))
                for p in range(NP):
                    nc.vector.tensor_copy(
                        vaug[p][:, kt, :, 0:64],
                        psv[:, 128 * p:128 * p + 128].rearrange(
                            "k (h d) -> k h d", h=2))
            return qm

        def attend_pair(qj, p, qm, vaug):
            """Attention for head pair p of q-block qj. Returns tp psum tile
            [128 v, 4 tq, 128 q]: transposed normalized attention output."""
            nki = 4 * qj + 4
            avh = [pp.tile([128, 2, 2, 65], F32, name=f"ps_av{half}",
                           tag="av", bufs=3)
                   for half in range(2)]
            tp = pp.tile([128, 4, 128], BF16, name="ps_tp", tag="av", bufs=3)
            # ot: normalized [q, (t, h, d)] bf16, one slot per qtile
            ot = otp.tile([128, 4, 2, 64], BF16, name="ot", tag="ot")

            def finish_half(half):
                # normalize: ot = av[...,0:64] * (1/den); den at col 64
                av = avh[half]
                rcp = smp.tile([128, 2, 2, 1], F32, name="rcp", tag="rcp")
                nc.vector.reciprocal(
                    rcp, av[:, :, :, 64:65])
                nc.vector.tensor_mul(
                    ot[:, 2 * half:2 * half + 2, :, :],
                    av[:, :, :, 0:64],
                    rcp.broadcast_to([128, 2, 2, 64]))
                # transpose the two qtiles of this half: [q, 2h x 64] -> [v, q]
                # all 4 transposes share one psum bank: single accumulation
                # group (start zeroes the whole 2KB zero region; disjoint
                # slots then accumulate onto zeros)
                for tq in range(2):
                    t = 2 * half + tq
                    nc.tensor.matmul(
                        tp[:, t, :],
                        ot[:, t, :, :].rearrange("q h d -> q (h d)"),
                        g["id"], is_transpose=True,
                        start=(t == 0), stop=(t == 3))

            for ki in range(nki):
                diag = (ki // 4 == qj)
                off = KB * (ki % 4) if diag else 0
                ksl = slice(ki * KB, (ki + 1) * KB)
                sc = pp.tile([128, 2, QB], F32, name="ps_sc", tag="sc", bufs=2)
                for h in range(2):
                    hp = slice(64 * h, 64 * h + 64)
                    nc.tensor.matmul(sc[:, h, off:], g["km"][p][hp, ksl],
                                     qm[hp, off:], start=True, stop=not diag,
                                     tile_position=(64 * h, 0))
                    if diag:
                        # accumulate -1e9 strict-lower-tri(k,q) causal mask:
                        # out += trimT^T @ I; exp then zeroes masked slots
                        nc.tensor.matmul(sc[:, h, off:off + KB], g["tri"],
                                         g["id"], start=False, stop=True)
                pt = ptp.tile([128, 2, QB], BF16, name="pt", tag="pt")
                nc.scalar.activation(out=pt[:, :, off:], in_=sc[:, :, off:],
                                     func=EXP, scale=0.125)
                t0 = max(0, ki - 4 * qj)
                for t in range(t0, 4):
                    half, tq = t // 2, t % 2
                    for h in range(2):
                        # one accumulation group per avh bank: first write
                        # zeroes the whole bank, last (its diag, h=1) stops
                        nc.tensor.matmul(
                            avh[half][:, tq, h, :],
                            pt[:, h, t * KB:(t + 1) * KB],
                            vaug[p][:, ki, h, :],
                            start=(ki == 0 and tq == 0 and h == 0),
                            stop=(ki == 4 * qj + 2 * half + 1 and tq == 1
                                  and h == 1))
                # halves complete as soon as their diagonal ki is done
                if diag and ki % 4 == 1:
                    finish_half(0)
            finish_half(1)
            return tp

        def out_block(qj, otT, wo_sb):
            """Output projection for q block qj from otT[p] = [128 v, 4 tq,
            128 q] bf16 sbuf tiles."""
            for t in range(4):
                pso = pp.tile([128, 2, QB], F32, name="ps_o", tag="sc", bufs=2)
                ost = osp.tile([128, 2, QB], BF16, name="ost", tag="ost")
                for dt_ in range(2):
                    for p in range(NP):
                        nc.tensor.matmul(
                            pso[:, dt_, :], otT[p][:, t, :],
                            wo_sb[:, p, dt_ * QB:(dt_ + 1) * QB],
                            start=(p == 0), stop=(p == NP - 1))
                    nc.vector.tensor_copy(ost[:, dt_, :], pso[:, dt_, :])
                nc.sync.dma_start(
                    out=out[qj * QB + t * KB: qj * QB + (t + 1) * KB, :],
                    in_=ost.rearrange("p a b -> p (a b)"))

        def attend_and_copy(st, p, qm, vaug):
            tp = attend_pair(st, p, qm, vaug)
            ott = otp.tile([128, 4, 128], BF16, name="otT", tag="otT")
            nc.vector.tensor_copy(ott, tp)
            return ott

        wo_box = []
        for _rep in range(repeat):
            vaug = [vap.tile([128, nk, 2, 65], BF16, name=f"vaug{p}",
                             tag=f"vaug{p}") for p in range(NP)]
            for p in range(NP):
                nc.vector.memset(vaug[p][:, :, :, 64], 1.0)
            xts = load_x(0)
            if _rep == 0:
                wo_sb_, cs_, sn_, tri_, id_ = load_rest_of_consts()
                g.update({"cs": cs_, "sn": sn_, "tri": tri_, "id": id_})
                wo_box.append(wo_sb_)
            wo_sb = wo_box[0]
            g["km"] = [qkp.tile([128, S], BF16, name=f"km{p}", tag=f"km{p}")
                       for p in range(NP)]
            qm = project_block(0, xts, vaug)
            pending_out = None
            for st in range(nq):
                otA = attend_and_copy(st, 0, qm[0], vaug)
                if pending_out is not None:
                    out_block(st - 1, pending_out, wo_sb)
                qm_next = None
                if st + 1 < nq:
                    xts = load_x(st + 1)
                    qm_next = project_block(st + 1, xts, vaug)
                otB = attend_and_copy(st, 1, qm[1], vaug)
                pending_out = [otA, otB]
                qm = qm_next
            out_block(nq - 1, pending_out, wo_sb)

    nc.compile()
    return nc


# ---------------- host-side helpers ----------------

def core_slices(core):
    """Global W-row index arrays for a core's sharded weight layout."""
    hg = core % 4
    heads = [4 * hg + h for h in range(HPC)]
    qe_rows = np.concatenate(
        [64 * g + 2 * np.arange(32) for g in heads])          # [128]
    qo_rows = qe_rows + 1
    v_rows = np.concatenate([64 * g + np.arange(64) for g in heads])  # [256]
    return heads, qe_rows, qo_rows, v_rows


def make_in_map(core, x, W_q, W_k, W_v, W_o, positions, theta, S,
                mm_dtype="bf16"):
    import ml_dtypes
    bf = ml_dtypes.bfloat16
    b = core // 4
    _, qe_rows, qo_rows, v_rows = core_slices(core)
    cT = lambda a: np.ascontiguousarray(a.astype(bf))
    pos = np.asarray(positions).astype(np.float32)
    inv_freq = np.float32(theta) ** (
        -np.arange(0, 32, dtype=np.float32) * np.float32(2.0 / DK))
    ang = pos[None, :] * inv_freq[:, None]          # [32, S]
    cosb = np.tile(np.cos(ang), (4, 1)).astype(np.float32)
    sinb = np.tile(np.sin(ang), (4, 1)).astype(np.float32)
    # mask lhsT: out[k,q] += trim[q,k] via matmul with identity rhs;
    # want -1e9 where k > q  ->  trim[q,k] = -1e9 for k > q (strict upper)
    trim = np.triu(np.full((KB, KB), -1e9, np.float32), 1)

    def pmajor(wt):   # [d, ncol] -> [128, (d//128)*ncol] partition-major
        d, ncol = wt.shape
        return wt.reshape(d // 128, 128, ncol).transpose(1, 0, 2).reshape(
            128, (d // 128) * ncol)

    f32c = lambda a: np.ascontiguousarray(np.asarray(a, dtype=np.float32))
    return {
        "xT": cT(np.asarray(x[b]).T),
        "wqeT": cT(pmajor(np.asarray(W_q)[qe_rows].T)),
        "wqoT": cT(pmajor(np.asarray(W_q)[qo_rows].T)),
        "wkeT": cT(pmajor(np.asarray(W_k)[qe_rows].T)),
        "wkoT": cT(pmajor(np.asarray(W_k)[qo_rows].T)),
        "wvT": cT(pmajor(np.asarray(W_v)[v_rows].T)),
        "woT": cT(pmajor(np.asarray(W_o)[:, v_rows].T)),
        "cosb": cT(cosb[:, :S]),
        "sinb": cT(sinb[:, :S]),
        "trim": cT(trim),
        "idnt": cT(np.eye(128, dtype=np.float32)),
    }


# ---------------- public entry point ----------------

S_FULL = 2048
MM_DTYPE = "bf16"
_NC_CACHE = {}


def _get_nc():
    if "nc" not in _NC_CACHE:
        _NC_CACHE["nc"] = build_nc(S=S_FULL, mm_dtype=MM_DTYPE)
    return _NC_CACHE["nc"]


def kernel(x, W_q, W_k, W_v, W_o, token_positions, max_seq_len, theta):
    from concourse import bass_utils

    x = np.asarray(x, dtype=np.float32)
    W_q = np.asarray(W_q, dtype=np.float32)
    W_k = np.asarray(W_k, dtype=np.float32)
    W_v = np.asarray(W_v, dtype=np.float32)
    W_o = np.asarray(W_o, dtype=np.float32)
    positions = np.asarray(token_positions)
    theta_f = float(np.asarray(theta))

    nc = _get_nc()
    in_maps = [
        make_in_map(c, x, W_q, W_k, W_v, W_o, positions, theta_f, S_FULL,
                    mm_dtype=MM_DTYPE)
        for c in range(8)
    ]
    res = bass_utils.run_bass_kernel_spmd(nc, in_maps, core_ids=list(range(8)))
    outs = [np.asarray(res.results[c]["out"], dtype=np.float32)
            for c in range(8)]
    full = np.empty((2, S_FULL, 1024), np.float32)
    for b in range(2):
        full[b] = np.sum([outs[4 * b + i] for i in range(4)], axis=0,
                         dtype=np.float32)
    return full
